# revision 50
# baseline (speedup 1.0000x reference)
"""AttentivePooling kernel for Trainium2, 8-core data-parallel over batch.

Full inputs:  q [16, 2048, 256] f32, a [16, 2048, 256] f32, U [256, 256] f32
Full outputs: (q_out [16, 256] f32, a_out [16, 256] f32)

Math per batch b:
    S = (q[b] @ U) @ a[b].T          # [T, T] scores
    G = tanh(S)
    g_q = softmax(max_j G)           # row maxes -> [T]
    g_a = softmax(max_i G)           # col maxes -> [T]
    q_out = g_q @ q[b]; a_out = g_a @ a[b]

tanh is monotonic, so row/col maxes are taken on raw S and tanh applied to
the 2048-long max vectors only.  S is computed in bf16 (fp32 PSUM accum);
the softmax and the weighted-sum stages run in fp32.

Per-core schedule (2 batches, no cross-core communication), tuned against
the TRN2 instruction cost model:
  - load q,a token-major fp32 (kept for fp32 weighted sums)
  - bf16 casts on Pool; 128x128 PE transposes (identity matmul) to
    feature-major qT/aT, drained from PSUM to bf16 SBUF
  - qUT = (q U)^T on PE
  - S tiles [128,1024] in PSUM -> bf16 SBUF (drains mostly on ACT),
    row-max via log-folding tensor_max chains (DVE 2x mode) + small
    reduce, col-max accumulators split DVE (low half) / Pool (high half)
  - col-max partition reduction: gpsimd partition_all_reduce + one
    scatter DMA into token-major [128,16]
  - tanh -> exp(+accum) -> ones-matmul partition sum -> reciprocal ->
    weights; fp32 PE weighted sums -> outputs
"""

import numpy as np

import concourse.bacc as bacc
import concourse.bass as bass
import concourse.bass_isa as bass_isa
import concourse.mybir as mybir
import concourse.tile as tile
from concourse.bass_utils import run_bass_kernel_spmd

FP32 = mybir.dt.float32
BF16 = mybir.dt.bfloat16

N_CORES = 8
B, T, D = 16, 2048, 256
NB = B // N_CORES          # batches per core
P = 128                    # partitions
TT = T // P                # 16 token tiles
DC = D // P                # 2 feature chunks
NEG = -3.0e38


def _emit(tc, ctx, q_d, a_d, U_d, eye_d, qo_d, ao_d):
    nc = tc.nc
    const = ctx.enter_context(tc.tile_pool(name="const", bufs=1))
    p1 = ctx.enter_context(tc.tile_pool(name="p1", bufs=1))
    p2 = ctx.enter_context(tc.tile_pool(name="p2", bufs=2))
    p3 = ctx.enter_context(tc.tile_pool(name="p3", bufs=3))
    ps_s = ctx.enter_context(tc.tile_pool(name="ps_s", bufs=2, space="PSUM"))
    ps_t = ctx.enter_context(tc.tile_pool(name="ps_t", bufs=2, space="PSUM"))
    ps_qu = ctx.enter_context(tc.tile_pool(name="ps_qu", bufs=1, space="PSUM"))
    ps_sm = ctx.enter_context(tc.tile_pool(name="ps_sm", bufs=1, space="PSUM"))
    dram = ctx.enter_context(tc.tile_pool(name="dram", bufs=2, space="DRAM"))

    # ---- per-core constants ----
    U_f32 = const.tile([P, DC, D], FP32)
    nc.sync.dma_start(U_f32[:], U_d.ap().rearrange("(c p) e -> p c e", p=P))
    U_bf = const.tile([P, DC, D], BF16)
    nc.vector.tensor_copy(U_bf[:], U_f32[:])
    ones = const.tile([P, P], FP32)
    nc.gpsimd.memset(ones[:], 1.0)
    eye = const.tile([P, P], BF16)
    nc.sync.dma_start(eye[:], eye_d.ap())

    def head_lc(b, st):
        qb = q_d.ap()[b].rearrange("(t p) d -> p t d", p=P)  # [128, 16, 256]
        ab = a_d.ap()[b].rearrange("(t p) d -> p t d", p=P)
        q_f32 = p2.tile([P, TT, D], FP32, tag="q_f32")
        a_f32 = p2.tile([P, TT, D], FP32, tag="a_f32")
        q_bf = p1.tile([P, TT, D], BF16, tag="q_bf")
        a_bf = p1.tile([P, TT, D], BF16, tag="a_bf")
        xs = {"q": (qb, q_f32, q_bf), "a": (ab, a_f32, a_bf)}

        def load(which, g):
            src, f32, _ = xs[which]
            sl = slice(g * 4, (g + 1) * 4)
            nc.sync.dma_start(f32[:, sl, :], src[:, sl, :])

        def cast(which, g):
            _, f32, bf = xs[which]
            sl = slice(g * 4, (g + 1) * 4)
            nc.gpsimd.tensor_copy(bf[:, sl, :], f32[:, sl, :])

        order = (
            (("q", 0), ("a", 0), ("q", 1), ("a", 1),
             ("a", 2), ("a", 3), ("q", 2), ("q", 3))
            if b == 0
            else (("q", 0), ("a", 0), ("q", 1), ("a", 1),
                  ("q", 2), ("a", 2), ("q", 3), ("a", 3))
        )
        for which, g in order:
            load(which, g)
        for which, g in order:
            cast(which, g)
        st.update(q_f32=q_f32, a_f32=a_f32, q_bf=q_bf, a_bf=a_bf)

    def head_tq(b, st):
        q_bf, a_bf = st["q_bf"], st["a_bf"]
        qT = p2.tile([P, DC, T], BF16, tag="qT")
        aT = p2.tile([P, DC, T], BF16, tag="aT")
        qUT = p2.tile([P, DC, T], BF16, tag="qUT")
        xs = {"q": (q_bf, qT), "a": (a_bf, aT)}
        drain = [0]

        def tp(which, c, th):
            # 8 transposed 128x128 blocks per 1-bank bf16 PSUM tile
            x_bf, xT = xs[which]
            ps = ps_t.tile([P, 8, P], BF16, tag="tp")
            for k in range(8):
                t = th * 8 + k
                nc.tensor.transpose(
                    ps[:, k, :], x_bf[:, t, c * P : (c + 1) * P], eye[:]
                )
            dst = xT[:, c, th * 8 * P : (th + 1) * 8 * P]
            if drain[0] % 2 == 0:    # DVE reads bf16 PSUM at 2x_1p rate
                nc.scalar.copy(dst, ps[:])
            else:
                nc.vector.tensor_copy(dst, ps[:])
            drain[0] += 1

        def qut(ec, ih):
            # qUT[e, i] = sum_d U[d, e] * qT[d, i]  (one PSUM bank per half)
            for i2 in range(2):
                ps = ps_qu.tile([P, 512], FP32, tag="qu")
                lo = ih * 1024 + i2 * 512
                for dc in range(DC):
                    nc.tensor.matmul(
                        ps[:],
                        U_bf[:, dc, ec * P : (ec + 1) * P],
                        qT[:, dc, lo : lo + 512],
                        start=(dc == 0),
                        stop=(dc == DC - 1),
                    )
                nc.scalar.copy(qUT[:, ec, lo : lo + 512], ps[:])

        if b == 0:
            # order so in-order engines unblock the first S tiles fastest:
            # S(t=0) needs all of aT, qT tiles 0-7, and the ih=0 qUT blocks
            tp("q", 0, 0)
            tp("q", 1, 0)
            qut(0, 0)
            qut(1, 0)
            for args in (("a", 0, 0), ("a", 1, 0), ("a", 0, 1), ("a", 1, 1)):
                tp(*args)
            tp("q", 0, 1)
            tp("q", 1, 1)
            qut(0, 1)
            qut(1, 1)
        else:
            for which in ("q", "a"):
                for c in range(DC):
                    for th in range(2):
                        tp(which, c, th)
            for ec in range(DC):
                for ih in range(2):
                    qut(ec, ih)

        ACC = p2.tile([P, T], BF16, tag="ACC")
        nc.gpsimd.memset(ACC[:], NEG)
        st.update(qUT=qUT, aT=aT, ACC=ACC)

    def sloop(b, st):
        # S tiles: PE -> PSUM -> bf16 SBUF; row-max via fold chain (DVE);
        # col-max accumulator chain on DVE (2x mode)
        qUT, aT, ACC = st["qUT"], st["aT"], st["ACC"]
        rowmax = p1.tile([P, TT], FP32, tag="rowmax")
        Rall = p1.tile([P, TT, P], BF16, tag="Rall")
        cm_dram = dram.tile([2, T], BF16, tag="cm")
        for t in range(TT):
            last = t == TT - 1
            if last:
                s_sb = p2.tile([P, 2, 1024], BF16, tag="s15", name="s15")
            else:
                s_sb = p3.tile([P, 2, 1024], BF16, tag="s_sb", name="s_sb")
            pss0 = ps_s.tile([P, 1024], FP32, tag="pss", name="pss0")
            pss1 = ps_s.tile([P, 1024], FP32, tag="pss", name="pss1")
            # ec-outer keeps each qUT weight block stationary across all
            # four N=512 matmuls (half the LDWEIGHTS of h-outer)
            for ec in range(DC):
                for h, pss in ((0, pss0), (1, pss1)):
                    for j2 in range(2):
                        nc.tensor.matmul(
                            pss[:, j2 * 512 : (j2 + 1) * 512],
                            qUT[:, ec, t * P : (t + 1) * P],
                            aT[:, ec, h * 1024 + j2 * 512 : h * 1024 + (j2 + 1) * 512],
                            start=(ec == 0),
                            stop=(ec == DC - 1),
                        )
            nc.scalar.copy(s_sb[:, 0, :], pss0[:])
            nc.scalar.copy(s_sb[:, 1, :], pss1[:])
            if not last:
                # col-max accumulate: one 2x-mode op over the full i-tile;
                # the last tile goes straight to a Pool all-reduce in tail()
                ACCv = ACC[:].rearrange("p (a b) -> p a b", a=2)
                nc.vector.tensor_max(ACCv, ACCv, s_sb[:])
            else:
                st["s15"] = s_sb
            if t == TT - 2:
                # ACC (tiles 0..14) is final: overlap its partition
                # all-reduce and DRAM repack with the last tile's compute
                cm_all = p1.tile([P, T], BF16, tag="cm_all")
                nc.gpsimd.partition_all_reduce(
                    cm_all[:], ACC[:], channels=P, reduce_op=bass_isa.ReduceOp.max
                )
                nc.sync.dma_start(cm_dram[0:1, :], cm_all[0:1, :])
                cmA = p1.tile([P, TT], BF16, tag="cmA")
                nc.sync.dma_start(
                    cmA[:], cm_dram[0:1, :].rearrange("o (t p) -> (o p) t", p=P)
                )
                st.update(cmA=cmA, cm_dram=cm_dram)
            # row max: fold 2048 -> 128 with 2x-mode tensor_max; final
            # reduce is batched over 8 i-tiles to amortize op overhead
            R = p3.tile([P, 1024], BF16, tag="R")
            nc.vector.tensor_max(R[:], s_sb[:, 0, :], s_sb[:, 1, :])
            for w in (512, 256):
                nc.vector.tensor_max(R[:, 0:w], R[:, 0:w], R[:, w : 2 * w])
            nc.vector.tensor_max(Rall[:, t, :], R[:, 0:128], R[:, 128:256])
            if t % 8 == 7:
                sl = slice(t - 7, t + 1)
                nc.vector.reduce_max(
                    rowmax[:, sl], Rall[:, sl, :], axis=mybir.AxisListType.X
                )
        st["rowmax"] = rowmax

    def tail(b, st):
        rowmax = st["rowmax"]
        q_f32, a_f32 = st["q_f32"], st["a_f32"]
        cmA, cm_dram, s15 = st["cmA"], st["cm_dram"], st["s15"]
        # last tile's col maxes: Pool all-reduce + DRAM repack, then merge
        # with the (already bounced) accumulator row
        scr = p1.tile([P, T], BF16, tag="scr15")
        nc.gpsimd.partition_all_reduce(
            scr[:],
            s15[:].rearrange("p a b -> p (a b)"),
            channels=P,
            reduce_op=bass_isa.ReduceOp.max,
        )
        nc.sync.dma_start(cm_dram[1:2, :], scr[0:1, :])
        cmB = p1.tile([P, TT], BF16, tag="cmB")
        nc.sync.dma_start(
            cmB[:], cm_dram[1:2, :].rearrange("o (t p) -> (o p) t", p=P)
        )
        colmax = p1.tile([P, TT], BF16, tag="colmax")
        nc.vector.tensor_max(colmax[:], cmA[:], cmB[:])

        # softmax weights; tanh bounds logits to [-1,1] so no max-shift
        smws = ps_sm.tile([P, 8], FP32, tag="smws")
        g_r = []
        for k, mx in enumerate((rowmax, colmax)):
            th_t = p1.tile([P, TT], FP32, tag=f"th{k}")
            nc.scalar.activation(th_t[:], mx[:], mybir.ActivationFunctionType.Tanh)
            ex = p1.tile([P, TT], FP32, tag=f"ex{k}")
            part = p1.tile([P, 1], FP32, tag=f"part{k}")
            nc.scalar.activation(
                ex[:], th_t[:], mybir.ActivationFunctionType.Exp, accum_out=part[:]
            )
            nc.tensor.matmul(
                smws[:, 4 + k : 5 + k], ones[:], part[:], start=True, stop=True
            )
            inv = p1.tile([P, 1], FP32, tag=f"inv{k}")
            nc.vector.reciprocal(inv[:], smws[:, 4 + k : 5 + k])
            g = p1.tile([P, TT], FP32, tag=f"g{k}")
            nc.vector.tensor_scalar_mul(g[:], ex[:], inv[:, 0:1])
            g_r.append(g)
        g_q, g_a = g_r

        # weighted sums: out[d] = sum_i g[i] * x[b, i, d]   (fp32 on PE)
        for k, (x_f32, g) in enumerate(((q_f32, g_q), (a_f32, g_a))):
            for dh in range(DC):
                col = k * DC + dh
                for t in range(TT):
                    nc.tensor.matmul(
                        smws[:, col : col + 1],
                        x_f32[:, t, dh * P : (dh + 1) * P],
                        g[:, t : t + 1],
                        start=(t == 0),
                        stop=(t == TT - 1),
                    )
        out_sb = p1.tile([P, 4], FP32, tag="out_sb")
        nc.scalar.copy(out_sb[:], smws[:, 0:4])
        nc.sync.dma_start(
            qo_d.ap()[b].rearrange("(c p) -> p c", p=P), out_sb[:, 0:DC]
        )
        nc.sync.dma_start(
            ao_d.ap()[b].rearrange("(c p) -> p c", p=P), out_sb[:, DC : 2 * DC]
        )

    # Emission order = scheduler priority on in-order engines: batch b+1's
    # loads/casts are emitted before batch b's S-loop (whose Pool all-
    # reduces would otherwise head-of-line block the casts), and batch b's
    # tail after batch b+1's transposes.
    sts = [{} for _ in range(NB)]
    head_lc(0, sts[0])
    head_tq(0, sts[0])
    for b in range(1, NB):
        head_lc(b, sts[b])
        sloop(b - 1, sts[b - 1])
        head_tq(b, sts[b])
        tail(b - 1, sts[b - 1])
    sloop(NB - 1, sts[NB - 1])
    tail(NB - 1, sts[NB - 1])


def build():
    nc = bacc.Bacc("TRN2", target_bir_lowering=False, debug=False)
    q_d = nc.declare_dram_parameter("q", [NB, T, D], FP32, isOutput=False)
    a_d = nc.declare_dram_parameter("a", [NB, T, D], FP32, isOutput=False)
    U_d = nc.declare_dram_parameter("U", [D, D], FP32, isOutput=False)
    eye_d = nc.declare_dram_parameter("eye", [P, P], BF16, isOutput=False)
    qo_d = nc.declare_dram_parameter("q_out", [NB, D], FP32, isOutput=True)
    ao_d = nc.declare_dram_parameter("a_out", [NB, D], FP32, isOutput=True)

    from contextlib import ExitStack

    with tile.TileContext(nc) as tc:
        with ExitStack() as ctx:
            _emit(tc, ctx, q_d, a_d, U_d, eye_d, qo_d, ao_d)
    nc.compile()
    return nc


_NC = None


def _get_nc():
    global _NC
    if _NC is None:
        _NC = build()
    return _NC


def make_in_maps(q, a, U):
    import ml_dtypes

    q = np.ascontiguousarray(q, dtype=np.float32)
    a = np.ascontiguousarray(a, dtype=np.float32)
    U = np.ascontiguousarray(U, dtype=np.float32)
    eye = np.eye(P, dtype=ml_dtypes.bfloat16)
    return [
        {
            "q": q[i * NB : (i + 1) * NB],
            "a": a[i * NB : (i + 1) * NB],
            "U": U,
            "eye": eye,
        }
        for i in range(N_CORES)
    ]


def kernel(q, a, U):
    nc = _get_nc()
    res = run_bass_kernel_spmd(nc, make_in_maps(q, a, U), list(range(N_CORES)))
    q_out = np.concatenate([r["q_out"] for r in res.results], axis=0)
    a_out = np.concatenate([r["a_out"] for r in res.results], axis=0)
    return (q_out, a_out)


# revision 51
# speedup vs baseline: 1.0096x; 1.0096x over previous
"""AttentivePooling kernel for Trainium2, 8-core data-parallel over batch.

Full inputs:  q [16, 2048, 256] f32, a [16, 2048, 256] f32, U [256, 256] f32
Full outputs: (q_out [16, 256] f32, a_out [16, 256] f32)

Math per batch b:
    S = (q[b] @ U) @ a[b].T          # [T, T] scores
    G = tanh(S)
    g_q = softmax(max_j G)           # row maxes -> [T]
    g_a = softmax(max_i G)           # col maxes -> [T]
    q_out = g_q @ q[b]; a_out = g_a @ a[b]

tanh is monotonic, so row/col maxes are taken on raw S and tanh applied to
the 2048-long max vectors only.  S is computed in bf16 (fp32 PSUM accum);
the softmax and the weighted-sum stages run in fp32.

Per-core schedule (2 batches, no cross-core communication), tuned against
the TRN2 instruction cost model:
  - load q,a token-major fp32 (kept for fp32 weighted sums)
  - bf16 casts on Pool; 128x128 PE transposes (identity matmul) to
    feature-major qT/aT, drained from PSUM to bf16 SBUF
  - qUT = (q U)^T on PE
  - S tiles [128,1024] in PSUM -> bf16 SBUF (drains mostly on ACT),
    row-max via log-folding tensor_max chains (DVE 2x mode) + small
    reduce, col-max accumulators split DVE (low half) / Pool (high half)
  - col-max partition reduction: gpsimd partition_all_reduce + one
    scatter DMA into token-major [128,16]
  - tanh -> exp(+accum) -> ones-matmul partition sum -> reciprocal ->
    weights; fp32 PE weighted sums -> outputs
"""

import numpy as np

import concourse.bacc as bacc
import concourse.bass as bass
import concourse.bass_isa as bass_isa
import concourse.mybir as mybir
import concourse.tile as tile
from concourse.bass_utils import run_bass_kernel_spmd

FP32 = mybir.dt.float32
BF16 = mybir.dt.bfloat16

N_CORES = 8
B, T, D = 16, 2048, 256
NB = B // N_CORES          # batches per core
P = 128                    # partitions
TT = T // P                # 16 token tiles
DC = D // P                # 2 feature chunks
NEG = -3.0e38


def _emit(tc, ctx, q_d, a_d, U_d, eye_d, qo_d, ao_d):
    nc = tc.nc
    const = ctx.enter_context(tc.tile_pool(name="const", bufs=1))
    p1 = ctx.enter_context(tc.tile_pool(name="p1", bufs=1))
    p2 = ctx.enter_context(tc.tile_pool(name="p2", bufs=2))
    p3 = ctx.enter_context(tc.tile_pool(name="p3", bufs=3))
    ps_s = ctx.enter_context(tc.tile_pool(name="ps_s", bufs=2, space="PSUM"))
    ps_t = ctx.enter_context(tc.tile_pool(name="ps_t", bufs=2, space="PSUM"))
    ps_qu = ctx.enter_context(tc.tile_pool(name="ps_qu", bufs=1, space="PSUM"))
    ps_sm = ctx.enter_context(tc.tile_pool(name="ps_sm", bufs=1, space="PSUM"))
    dram = ctx.enter_context(tc.tile_pool(name="dram", bufs=2, space="DRAM"))

    # ---- per-core constants ----
    U_f32 = const.tile([P, DC, D], FP32)
    nc.sync.dma_start(U_f32[:], U_d.ap().rearrange("(c p) e -> p c e", p=P))
    U_bf = const.tile([P, DC, D], BF16)
    nc.vector.tensor_copy(U_bf[:], U_f32[:])
    ones = const.tile([P, P], FP32)
    nc.gpsimd.memset(ones[:], 1.0)
    eye = const.tile([P, P], BF16)
    nc.sync.dma_start(eye[:], eye_d.ap())

    def head_lc(b, st):
        qb = q_d.ap()[b].rearrange("(t p) d -> p t d", p=P)  # [128, 16, 256]
        ab = a_d.ap()[b].rearrange("(t p) d -> p t d", p=P)
        q_f32 = p2.tile([P, TT, D], FP32, tag="q_f32")
        a_f32 = p2.tile([P, TT, D], FP32, tag="a_f32")
        q_bf = p1.tile([P, TT, D], BF16, tag="q_bf")
        a_bf = p1.tile([P, TT, D], BF16, tag="a_bf")
        xs = {"q": (qb, q_f32, q_bf), "a": (ab, a_f32, a_bf)}

        def load(which, g):
            src, f32, _ = xs[which]
            sl = slice(g * 4, (g + 1) * 4)
            nc.sync.dma_start(f32[:, sl, :], src[:, sl, :])

        def cast(which, g):
            _, f32, bf = xs[which]
            sl = slice(g * 4, (g + 1) * 4)
            nc.gpsimd.tensor_copy(bf[:, sl, :], f32[:, sl, :])

        order = (
            (("q", 0), ("a", 0), ("q", 1), ("a", 1),
             ("a", 2), ("a", 3), ("q", 2), ("q", 3))
            if b == 0
            else (("q", 0), ("a", 0), ("q", 1), ("a", 1),
                  ("q", 2), ("a", 2), ("q", 3), ("a", 3))
        )
        for which, g in order:
            load(which, g)
        for which, g in order:
            cast(which, g)
        st.update(q_f32=q_f32, a_f32=a_f32, q_bf=q_bf, a_bf=a_bf)

    def head_tq(b, st):
        q_bf, a_bf = st["q_bf"], st["a_bf"]
        qT = p2.tile([P, DC, T], BF16, tag="qT")
        aT = p2.tile([P, DC, T], BF16, tag="aT")
        qUT = p2.tile([P, DC, T], BF16, tag="qUT")
        xs = {"q": (q_bf, qT), "a": (a_bf, aT)}
        drain = [0]

        def tp(which, c, th):
            # 8 transposed 128x128 blocks per 1-bank bf16 PSUM tile
            x_bf, xT = xs[which]
            ps = ps_t.tile([P, 8, P], BF16, tag="tp")
            for k in range(8):
                t = th * 8 + k
                nc.tensor.transpose(
                    ps[:, k, :], x_bf[:, t, c * P : (c + 1) * P], eye[:]
                )
            dst = xT[:, c, th * 8 * P : (th + 1) * 8 * P]
            if drain[0] % 2 == 0:    # DVE reads bf16 PSUM at 2x_1p rate
                nc.scalar.copy(dst, ps[:])
            else:
                nc.vector.tensor_copy(dst, ps[:])
            drain[0] += 1

        def qut(ec, ih):
            # qUT[e, i] = sum_d U[d, e] * qT[d, i]  (one PSUM bank per half)
            for i2 in range(2):
                ps = ps_qu.tile([P, 512], FP32, tag="qu")
                lo = ih * 1024 + i2 * 512
                for dc in range(DC):
                    nc.tensor.matmul(
                        ps[:],
                        U_bf[:, dc, ec * P : (ec + 1) * P],
                        qT[:, dc, lo : lo + 512],
                        start=(dc == 0),
                        stop=(dc == DC - 1),
                    )
                nc.scalar.copy(qUT[:, ec, lo : lo + 512], ps[:])

        if b == 0:
            # order so in-order engines unblock the first S tiles fastest:
            # S(t=0) needs all of aT, qT tiles 0-7, and the ih=0 qUT blocks
            tp("q", 0, 0)
            tp("q", 1, 0)
            qut(0, 0)
            qut(1, 0)
            for args in (("a", 0, 0), ("a", 1, 0), ("a", 0, 1), ("a", 1, 1)):
                tp(*args)
            tp("q", 0, 1)
            tp("q", 1, 1)
            qut(0, 1)
            qut(1, 1)
        else:
            for which in ("q", "a"):
                for c in range(DC):
                    for th in range(2):
                        tp(which, c, th)
            for ec in range(DC):
                for ih in range(2):
                    qut(ec, ih)

        ACC = p2.tile([P, T], BF16, tag="ACC")
        nc.gpsimd.memset(ACC[:], NEG)
        st.update(qUT=qUT, aT=aT, ACC=ACC)

    def sloop(b, st):
        # S tiles: PE -> PSUM -> bf16 SBUF; row-max via fold chain (DVE);
        # col-max accumulator chain on DVE (2x mode)
        qUT, aT, ACC = st["qUT"], st["aT"], st["ACC"]
        rowmax = p1.tile([P, TT], FP32, tag="rowmax")
        Rall = p1.tile([P, TT, P], BF16, tag="Rall")
        cm_dram = dram.tile([2, T], BF16, tag="cm")
        for t in range(TT):
            last = t == TT - 1
            if last:
                s_sb = p2.tile([P, 2, 1024], BF16, tag="s15", name="s15")
            else:
                s_sb = p3.tile([P, 2, 1024], BF16, tag="s_sb", name="s_sb")
            pss0 = ps_s.tile([P, 1024], FP32, tag="pss", name="pss0")
            pss1 = ps_s.tile([P, 1024], FP32, tag="pss", name="pss1")
            # ec-outer keeps each qUT weight block stationary across all
            # four N=512 matmuls (half the LDWEIGHTS of h-outer)
            for ec in range(DC):
                for h, pss in ((0, pss0), (1, pss1)):
                    for j2 in range(2):
                        nc.tensor.matmul(
                            pss[:, j2 * 512 : (j2 + 1) * 512],
                            qUT[:, ec, t * P : (t + 1) * P],
                            aT[:, ec, h * 1024 + j2 * 512 : h * 1024 + (j2 + 1) * 512],
                            start=(ec == 0),
                            stop=(ec == DC - 1),
                        )
            nc.scalar.copy(s_sb[:, 0, :], pss0[:])
            nc.scalar.copy(s_sb[:, 1, :], pss1[:])
            if not last:
                # col-max accumulate: one 2x-mode op over the full i-tile;
                # the last tile goes straight to a Pool all-reduce in tail()
                ACCv = ACC[:].rearrange("p (a b) -> p a b", a=2)
                nc.vector.tensor_max(ACCv, ACCv, s_sb[:])
            else:
                st["s15"] = s_sb
            if t == TT - 2:
                # ACC (tiles 0..14) is final: overlap its partition
                # all-reduce and DRAM repack with the last tile's compute
                cm_all = p1.tile([P, T], BF16, tag="cm_all")
                nc.gpsimd.partition_all_reduce(
                    cm_all[:], ACC[:], channels=P, reduce_op=bass_isa.ReduceOp.max
                )
                nc.sync.dma_start(cm_dram[0:1, :], cm_all[0:1, :])
                cmA = p1.tile([P, TT], BF16, tag="cmA")
                nc.sync.dma_start(
                    cmA[:], cm_dram[0:1, :].rearrange("o (t p) -> (o p) t", p=P)
                )
                st.update(cmA=cmA, cm_dram=cm_dram)
            # row max: fold 2048 -> 128 with 2x-mode tensor_max; final
            # reduce is batched over 8 i-tiles to amortize op overhead
            R = p3.tile([P, 1024], BF16, tag="R")
            nc.vector.tensor_max(R[:], s_sb[:, 0, :], s_sb[:, 1, :])
            for w in (512, 256):
                nc.vector.tensor_max(R[:, 0:w], R[:, 0:w], R[:, w : 2 * w])
            nc.vector.tensor_max(Rall[:, t, :], R[:, 0:128], R[:, 128:256])
            if t % 8 == 7:
                sl = slice(t - 7, t + 1)
                nc.vector.reduce_max(
                    rowmax[:, sl], Rall[:, sl, :], axis=mybir.AxisListType.X
                )
        st["rowmax"] = rowmax

    def tail(b, st):
        rowmax = st["rowmax"]
        q_f32, a_f32 = st["q_f32"], st["a_f32"]
        cmA, cm_dram, s15 = st["cmA"], st["cm_dram"], st["s15"]
        # last tile's col maxes, merged with the (already bounced)
        # accumulator row
        cmB = p1.tile([P, TT], BF16, tag="cmB")
        s15v = s15[:].rearrange("p a b -> p (a b)")
        if b == NB - 1:
            # kernel tail: PE is idle, so transpose tile 15 and reduce on
            # DVE directly -- skips the Pool all-reduce + DRAM round trip
            for grp in range(2):
                ps = ps_t.tile([P, 8, P], BF16, tag="tp", name="tptail")
                for k in range(8):
                    blk = grp * 8 + k
                    nc.tensor.transpose(
                        ps[:, k, :], s15v[:, blk * P : (blk + 1) * P], eye[:]
                    )
                nc.vector.reduce_max(
                    cmB[:, grp * 8 : (grp + 1) * 8],
                    ps[:],
                    axis=mybir.AxisListType.X,
                )
        else:
            scr = p1.tile([P, T], BF16, tag="scr15")
            nc.gpsimd.partition_all_reduce(
                scr[:], s15v, channels=P, reduce_op=bass_isa.ReduceOp.max
            )
            nc.sync.dma_start(cm_dram[1:2, :], scr[0:1, :])
            nc.sync.dma_start(
                cmB[:], cm_dram[1:2, :].rearrange("o (t p) -> (o p) t", p=P)
            )
        colmax = p1.tile([P, TT], BF16, tag="colmax")
        nc.vector.tensor_max(colmax[:], cmA[:], cmB[:])

        # softmax weights; tanh bounds logits to [-1,1] so no max-shift
        smws = ps_sm.tile([P, 8], FP32, tag="smws")
        g_r = []
        for k, mx in enumerate((rowmax, colmax)):
            th_t = p1.tile([P, TT], FP32, tag=f"th{k}")
            nc.scalar.activation(th_t[:], mx[:], mybir.ActivationFunctionType.Tanh)
            ex = p1.tile([P, TT], FP32, tag=f"ex{k}")
            part = p1.tile([P, 1], FP32, tag=f"part{k}")
            nc.scalar.activation(
                ex[:], th_t[:], mybir.ActivationFunctionType.Exp, accum_out=part[:]
            )
            nc.tensor.matmul(
                smws[:, 4 + k : 5 + k], ones[:], part[:], start=True, stop=True
            )
            inv = p1.tile([P, 1], FP32, tag=f"inv{k}")
            nc.vector.reciprocal(inv[:], smws[:, 4 + k : 5 + k])
            g = p1.tile([P, TT], FP32, tag=f"g{k}")
            nc.vector.tensor_scalar_mul(g[:], ex[:], inv[:, 0:1])
            g_r.append(g)
        g_q, g_a = g_r

        # weighted sums: out[d] = sum_i g[i] * x[b, i, d]   (fp32 on PE)
        for k, (x_f32, g) in enumerate(((q_f32, g_q), (a_f32, g_a))):
            for dh in range(DC):
                col = k * DC + dh
                for t in range(TT):
                    nc.tensor.matmul(
                        smws[:, col : col + 1],
                        x_f32[:, t, dh * P : (dh + 1) * P],
                        g[:, t : t + 1],
                        start=(t == 0),
                        stop=(t == TT - 1),
                    )
        out_sb = p1.tile([P, 4], FP32, tag="out_sb")
        nc.scalar.copy(out_sb[:], smws[:, 0:4])
        nc.sync.dma_start(
            qo_d.ap()[b].rearrange("(c p) -> p c", p=P), out_sb[:, 0:DC]
        )
        nc.sync.dma_start(
            ao_d.ap()[b].rearrange("(c p) -> p c", p=P), out_sb[:, DC : 2 * DC]
        )

    # Emission order = scheduler priority on in-order engines: batch b+1's
    # loads/casts are emitted before batch b's S-loop (whose Pool all-
    # reduces would otherwise head-of-line block the casts), and batch b's
    # tail after batch b+1's transposes.
    sts = [{} for _ in range(NB)]
    head_lc(0, sts[0])
    head_tq(0, sts[0])
    for b in range(1, NB):
        head_lc(b, sts[b])
        sloop(b - 1, sts[b - 1])
        head_tq(b, sts[b])
        tail(b - 1, sts[b - 1])
    sloop(NB - 1, sts[NB - 1])
    tail(NB - 1, sts[NB - 1])


def build():
    nc = bacc.Bacc("TRN2", target_bir_lowering=False, debug=False)
    q_d = nc.declare_dram_parameter("q", [NB, T, D], FP32, isOutput=False)
    a_d = nc.declare_dram_parameter("a", [NB, T, D], FP32, isOutput=False)
    U_d = nc.declare_dram_parameter("U", [D, D], FP32, isOutput=False)
    eye_d = nc.declare_dram_parameter("eye", [P, P], BF16, isOutput=False)
    qo_d = nc.declare_dram_parameter("q_out", [NB, D], FP32, isOutput=True)
    ao_d = nc.declare_dram_parameter("a_out", [NB, D], FP32, isOutput=True)

    from contextlib import ExitStack

    with tile.TileContext(nc) as tc:
        with ExitStack() as ctx:
            _emit(tc, ctx, q_d, a_d, U_d, eye_d, qo_d, ao_d)
    nc.compile()
    return nc


_NC = None


def _get_nc():
    global _NC
    if _NC is None:
        _NC = build()
    return _NC


def make_in_maps(q, a, U):
    import ml_dtypes

    q = np.ascontiguousarray(q, dtype=np.float32)
    a = np.ascontiguousarray(a, dtype=np.float32)
    U = np.ascontiguousarray(U, dtype=np.float32)
    eye = np.eye(P, dtype=ml_dtypes.bfloat16)
    return [
        {
            "q": q[i * NB : (i + 1) * NB],
            "a": a[i * NB : (i + 1) * NB],
            "U": U,
            "eye": eye,
        }
        for i in range(N_CORES)
    ]


def kernel(q, a, U):
    nc = _get_nc()
    res = run_bass_kernel_spmd(nc, make_in_maps(q, a, U), list(range(N_CORES)))
    q_out = np.concatenate([r["q_out"] for r in res.results], axis=0)
    a_out = np.concatenate([r["a_out"] for r in res.results], axis=0)
    return (q_out, a_out)


# revision 54
# speedup vs baseline: 1.0177x; 1.0081x over previous
"""AttentivePooling kernel for Trainium2, 8-core data-parallel over batch.

Full inputs:  q [16, 2048, 256] f32, a [16, 2048, 256] f32, U [256, 256] f32
Full outputs: (q_out [16, 256] f32, a_out [16, 256] f32)

Math per batch b:
    S = (q[b] @ U) @ a[b].T          # [T, T] scores
    G = tanh(S)
    g_q = softmax(max_j G)           # row maxes -> [T]
    g_a = softmax(max_i G)           # col maxes -> [T]
    q_out = g_q @ q[b]; a_out = g_a @ a[b]

tanh is monotonic, so row/col maxes are taken on raw S and tanh applied to
the 2048-long max vectors only.  S is computed in bf16 (fp32 PSUM accum);
the softmax and the weighted-sum stages run in fp32.

Per-core schedule (2 batches, no cross-core communication), tuned against
the TRN2 instruction cost model:
  - load q,a token-major fp32 (kept for fp32 weighted sums)
  - bf16 casts on Pool; 128x128 PE transposes (identity matmul) to
    feature-major qT/aT, drained from PSUM to bf16 SBUF
  - qUT = (q U)^T on PE
  - S tiles [128,1024] in PSUM -> bf16 SBUF (drains mostly on ACT),
    row-max via log-folding tensor_max chains (DVE 2x mode) + small
    reduce, col-max accumulators split DVE (low half) / Pool (high half)
  - col-max partition reduction: gpsimd partition_all_reduce + one
    scatter DMA into token-major [128,16]
  - tanh -> exp(+accum) -> ones-matmul partition sum -> reciprocal ->
    weights; fp32 PE weighted sums -> outputs
"""

import numpy as np

import concourse.bacc as bacc
import concourse.bass as bass
import concourse.bass_isa as bass_isa
import concourse.mybir as mybir
import concourse.tile as tile
from concourse.bass_utils import run_bass_kernel_spmd

FP32 = mybir.dt.float32
BF16 = mybir.dt.bfloat16

N_CORES = 8
B, T, D = 16, 2048, 256
NB = B // N_CORES          # batches per core
P = 128                    # partitions
TT = T // P                # 16 token tiles
DC = D // P                # 2 feature chunks
NEG = -3.0e38


def _emit(tc, ctx, q_d, a_d, U_d, eye_d, qo_d, ao_d):
    nc = tc.nc
    const = ctx.enter_context(tc.tile_pool(name="const", bufs=1))
    p1 = ctx.enter_context(tc.tile_pool(name="p1", bufs=1))
    p2 = ctx.enter_context(tc.tile_pool(name="p2", bufs=2))
    p3 = ctx.enter_context(tc.tile_pool(name="p3", bufs=3))
    ps_s = ctx.enter_context(tc.tile_pool(name="ps_s", bufs=2, space="PSUM"))
    ps_t = ctx.enter_context(tc.tile_pool(name="ps_t", bufs=2, space="PSUM"))
    ps_qu = ctx.enter_context(tc.tile_pool(name="ps_qu", bufs=1, space="PSUM"))
    ps_sm = ctx.enter_context(tc.tile_pool(name="ps_sm", bufs=1, space="PSUM"))
    dram = ctx.enter_context(tc.tile_pool(name="dram", bufs=2, space="DRAM"))

    # ---- per-core constants ----
    U_f32 = const.tile([P, DC, D], FP32)
    nc.sync.dma_start(U_f32[:], U_d.ap().rearrange("(c p) e -> p c e", p=P))
    U_bf = const.tile([P, DC, D], BF16)
    nc.vector.tensor_copy(U_bf[:], U_f32[:])
    ones = const.tile([P, P], FP32)
    nc.gpsimd.memset(ones[:], 1.0)
    eye = const.tile([P, P], BF16)
    nc.sync.dma_start(eye[:], eye_d.ap())

    def head_lc(b, st):
        qb = q_d.ap()[b].rearrange("(t p) d -> p t d", p=P)  # [128, 16, 256]
        ab = a_d.ap()[b].rearrange("(t p) d -> p t d", p=P)
        q_f32 = p2.tile([P, TT, D], FP32, tag="q_f32")
        a_f32 = p2.tile([P, TT, D], FP32, tag="a_f32")
        q_bf = p1.tile([P, TT, D], BF16, tag="q_bf")
        a_bf = p1.tile([P, TT, D], BF16, tag="a_bf")
        xs = {"q": (qb, q_f32, q_bf), "a": (ab, a_f32, a_bf)}

        def load(which, g):
            # 8-tile (1MB) halves: fewer HWDGE issue slots puts the whole
            # batch in flight sooner
            src, f32, _ = xs[which]
            sl = slice(g * 8, (g + 1) * 8)
            nc.sync.dma_start(f32[:, sl, :], src[:, sl, :])

        def cast(which, g):
            _, f32, bf = xs[which]
            sl = slice(g * 4, (g + 1) * 4)
            nc.gpsimd.tensor_copy(bf[:, sl, :], f32[:, sl, :])

        lorder = (
            (("q", 0), ("a", 0), ("a", 1), ("q", 1))
            if b == 0
            else (("q", 0), ("a", 0), ("q", 1), ("a", 1))
        )
        for which, g in lorder:
            load(which, g)
        corder = (
            (("q", 0), ("a", 0), ("q", 1), ("a", 1),
             ("a", 2), ("a", 3), ("q", 2), ("q", 3))
            if b == 0
            else (("q", 0), ("a", 0), ("q", 1), ("a", 1),
                  ("q", 2), ("a", 2), ("q", 3), ("a", 3))
        )
        for which, g in corder:
            cast(which, g)
        st.update(q_f32=q_f32, a_f32=a_f32, q_bf=q_bf, a_bf=a_bf)

    def head_tq(b, st):
        q_bf, a_bf = st["q_bf"], st["a_bf"]
        qT = p2.tile([P, DC, T], BF16, tag="qT")
        aT = p2.tile([P, DC, T], BF16, tag="aT")
        qUT = p2.tile([P, DC, T], BF16, tag="qUT")
        xs = {"q": (q_bf, qT), "a": (a_bf, aT)}
        drain = [0]

        def tp(which, c, th):
            # 8 transposed 128x128 blocks per 1-bank bf16 PSUM tile
            x_bf, xT = xs[which]
            ps = ps_t.tile([P, 8, P], BF16, tag="tp")
            for k in range(8):
                t = th * 8 + k
                nc.tensor.transpose(
                    ps[:, k, :], x_bf[:, t, c * P : (c + 1) * P], eye[:]
                )
            dst = xT[:, c, th * 8 * P : (th + 1) * 8 * P]
            if drain[0] % 2 == 0:    # DVE reads bf16 PSUM at 2x_1p rate
                nc.scalar.copy(dst, ps[:])
            else:
                nc.vector.tensor_copy(dst, ps[:])
            drain[0] += 1

        def qut(ec, ih):
            # qUT[e, i] = sum_d U[d, e] * qT[d, i]  (one PSUM bank per half)
            for i2 in range(2):
                ps = ps_qu.tile([P, 512], FP32, tag="qu")
                lo = ih * 1024 + i2 * 512
                for dc in range(DC):
                    nc.tensor.matmul(
                        ps[:],
                        U_bf[:, dc, ec * P : (ec + 1) * P],
                        qT[:, dc, lo : lo + 512],
                        start=(dc == 0),
                        stop=(dc == DC - 1),
                    )
                nc.scalar.copy(qUT[:, ec, lo : lo + 512], ps[:])

        if b == 0:
            # order so in-order engines unblock the first S tiles fastest:
            # S(t=0) needs all of aT, qT tiles 0-7, and the ih=0 qUT blocks
            tp("q", 0, 0)
            tp("q", 1, 0)
            qut(0, 0)
            qut(1, 0)
            for args in (("a", 0, 0), ("a", 1, 0), ("a", 0, 1), ("a", 1, 1)):
                tp(*args)
            tp("q", 0, 1)
            tp("q", 1, 1)
            qut(0, 1)
            qut(1, 1)
        else:
            for which in ("q", "a"):
                for c in range(DC):
                    for th in range(2):
                        tp(which, c, th)
            for ec in range(DC):
                for ih in range(2):
                    qut(ec, ih)

        ACC = p2.tile([P, T], BF16, tag="ACC")
        nc.gpsimd.memset(ACC[:], NEG)
        st.update(qUT=qUT, aT=aT, ACC=ACC)

    def sloop(b, st):
        # S tiles: PE -> PSUM -> bf16 SBUF; row-max via fold chain (DVE);
        # col-max accumulator chain on DVE (2x mode)
        qUT, aT, ACC = st["qUT"], st["aT"], st["ACC"]
        rowmax = p1.tile([P, TT], FP32, tag="rowmax")
        Rall = p1.tile([P, TT, P], BF16, tag="Rall")
        cm_dram = dram.tile([2, T], BF16, tag="cm")
        for t in range(TT):
            last = t == TT - 1
            if last:
                s_sb = p2.tile([P, 2, 1024], BF16, tag="s15", name="s15")
            else:
                s_sb = p3.tile([P, 2, 1024], BF16, tag="s_sb", name="s_sb")
            pss0 = ps_s.tile([P, 1024], FP32, tag="pss", name="pss0")
            pss1 = ps_s.tile([P, 1024], FP32, tag="pss", name="pss1")
            # ec-outer keeps each qUT weight block stationary across all
            # four N=512 matmuls (half the LDWEIGHTS of h-outer)
            for ec in range(DC):
                for h, pss in ((0, pss0), (1, pss1)):
                    for j2 in range(2):
                        nc.tensor.matmul(
                            pss[:, j2 * 512 : (j2 + 1) * 512],
                            qUT[:, ec, t * P : (t + 1) * P],
                            aT[:, ec, h * 1024 + j2 * 512 : h * 1024 + (j2 + 1) * 512],
                            start=(ec == 0),
                            stop=(ec == DC - 1),
                        )
            nc.scalar.copy(s_sb[:, 0, :], pss0[:])
            nc.scalar.copy(s_sb[:, 1, :], pss1[:])
            if not last:
                # col-max accumulate: one 2x-mode op over the full i-tile;
                # the last tile goes straight to a Pool all-reduce in tail()
                ACCv = ACC[:].rearrange("p (a b) -> p a b", a=2)
                nc.vector.tensor_max(ACCv, ACCv, s_sb[:])
            else:
                st["s15"] = s_sb
            if t == TT - 2:
                # ACC (tiles 0..14) is final: overlap its partition
                # all-reduce and DRAM repack with the last tile's compute
                cm_all = p1.tile([P, T], BF16, tag="cm_all")
                nc.gpsimd.partition_all_reduce(
                    cm_all[:], ACC[:], channels=P, reduce_op=bass_isa.ReduceOp.max
                )
                nc.sync.dma_start(cm_dram[0:1, :], cm_all[0:1, :])
                cmA = p1.tile([P, TT], BF16, tag="cmA")
                nc.sync.dma_start(
                    cmA[:], cm_dram[0:1, :].rearrange("o (t p) -> (o p) t", p=P)
                )
                st.update(cmA=cmA, cm_dram=cm_dram)
            # row max: fold 2048 -> 128 with 2x-mode tensor_max; final
            # reduce is batched over 8 i-tiles to amortize op overhead
            R = p3.tile([P, 1024], BF16, tag="R")
            nc.vector.tensor_max(R[:], s_sb[:, 0, :], s_sb[:, 1, :])
            for w in (512, 256):
                nc.vector.tensor_max(R[:, 0:w], R[:, 0:w], R[:, w : 2 * w])
            nc.vector.tensor_max(Rall[:, t, :], R[:, 0:128], R[:, 128:256])
            if t % 8 == 7:
                sl = slice(t - 7, t + 1)
                nc.vector.reduce_max(
                    rowmax[:, sl], Rall[:, sl, :], axis=mybir.AxisListType.X
                )
        st["rowmax"] = rowmax

    def tail(b, st):
        rowmax = st["rowmax"]
        q_f32, a_f32 = st["q_f32"], st["a_f32"]
        cmA, cm_dram, s15 = st["cmA"], st["cm_dram"], st["s15"]
        # last tile's col maxes, merged with the (already bounced)
        # accumulator row
        cmB = p1.tile([P, TT], BF16, tag="cmB")
        s15v = s15[:].rearrange("p a b -> p (a b)")
        if b == NB - 1:
            # kernel tail: PE is idle, so transpose tile 15 and reduce on
            # DVE directly -- skips the Pool all-reduce + DRAM round trip
            for grp in range(2):
                ps = ps_t.tile([P, 8, P], BF16, tag="tp", name="tptail")
                for k in range(8):
                    blk = grp * 8 + k
                    nc.tensor.transpose(
                        ps[:, k, :], s15v[:, blk * P : (blk + 1) * P], eye[:]
                    )
                nc.vector.reduce_max(
                    cmB[:, grp * 8 : (grp + 1) * 8],
                    ps[:],
                    axis=mybir.AxisListType.X,
                )
        else:
            scr = p1.tile([P, T], BF16, tag="scr15")
            nc.gpsimd.partition_all_reduce(
                scr[:], s15v, channels=P, reduce_op=bass_isa.ReduceOp.max
            )
            nc.sync.dma_start(cm_dram[1:2, :], scr[0:1, :])
            nc.sync.dma_start(
                cmB[:], cm_dram[1:2, :].rearrange("o (t p) -> (o p) t", p=P)
            )
        colmax = p1.tile([P, TT], BF16, tag="colmax")
        nc.vector.tensor_max(colmax[:], cmA[:], cmB[:])

        # softmax weights; tanh bounds logits to [-1,1] so no max-shift
        smws = ps_sm.tile([P, 8], FP32, tag="smws")
        g_r = []
        for k, mx in enumerate((rowmax, colmax)):
            th_t = p1.tile([P, TT], FP32, tag=f"th{k}")
            nc.scalar.activation(th_t[:], mx[:], mybir.ActivationFunctionType.Tanh)
            ex = p1.tile([P, TT], FP32, tag=f"ex{k}")
            part = p1.tile([P, 1], FP32, tag=f"part{k}")
            nc.scalar.activation(
                ex[:], th_t[:], mybir.ActivationFunctionType.Exp, accum_out=part[:]
            )
            nc.tensor.matmul(
                smws[:, 4 + k : 5 + k], ones[:], part[:], start=True, stop=True
            )
            inv = p1.tile([P, 1], FP32, tag=f"inv{k}")
            nc.vector.reciprocal(inv[:], smws[:, 4 + k : 5 + k])
            g = p1.tile([P, TT], FP32, tag=f"g{k}")
            nc.vector.tensor_scalar_mul(g[:], ex[:], inv[:, 0:1])
            g_r.append(g)
        g_q, g_a = g_r

        # weighted sums: out[d] = sum_i g[i] * x[b, i, d]   (fp32 on PE)
        for k, (x_f32, g) in enumerate(((q_f32, g_q), (a_f32, g_a))):
            for dh in range(DC):
                col = k * DC + dh
                for t in range(TT):
                    nc.tensor.matmul(
                        smws[:, col : col + 1],
                        x_f32[:, t, dh * P : (dh + 1) * P],
                        g[:, t : t + 1],
                        start=(t == 0),
                        stop=(t == TT - 1),
                    )
        out_sb = p1.tile([P, 4], FP32, tag="out_sb")
        nc.scalar.copy(out_sb[:], smws[:, 0:4])
        nc.sync.dma_start(
            qo_d.ap()[b].rearrange("(c p) -> p c", p=P), out_sb[:, 0:DC]
        )
        nc.sync.dma_start(
            ao_d.ap()[b].rearrange("(c p) -> p c", p=P), out_sb[:, DC : 2 * DC]
        )

    # Emission order = scheduler priority on in-order engines: batch b+1's
    # loads/casts are emitted before batch b's S-loop (whose Pool all-
    # reduces would otherwise head-of-line block the casts), and batch b's
    # tail after batch b+1's transposes.
    sts = [{} for _ in range(NB)]
    head_lc(0, sts[0])
    head_tq(0, sts[0])
    for b in range(1, NB):
        head_lc(b, sts[b])
        sloop(b - 1, sts[b - 1])
        head_tq(b, sts[b])
        tail(b - 1, sts[b - 1])
    sloop(NB - 1, sts[NB - 1])
    tail(NB - 1, sts[NB - 1])


def build():
    nc = bacc.Bacc("TRN2", target_bir_lowering=False, debug=False)
    q_d = nc.declare_dram_parameter("q", [NB, T, D], FP32, isOutput=False)
    a_d = nc.declare_dram_parameter("a", [NB, T, D], FP32, isOutput=False)
    U_d = nc.declare_dram_parameter("U", [D, D], FP32, isOutput=False)
    eye_d = nc.declare_dram_parameter("eye", [P, P], BF16, isOutput=False)
    qo_d = nc.declare_dram_parameter("q_out", [NB, D], FP32, isOutput=True)
    ao_d = nc.declare_dram_parameter("a_out", [NB, D], FP32, isOutput=True)

    from contextlib import ExitStack

    with tile.TileContext(nc) as tc:
        with ExitStack() as ctx:
            _emit(tc, ctx, q_d, a_d, U_d, eye_d, qo_d, ao_d)
    nc.compile()
    return nc


_NC = None


def _get_nc():
    global _NC
    if _NC is None:
        _NC = build()
    return _NC


def make_in_maps(q, a, U):
    import ml_dtypes

    q = np.ascontiguousarray(q, dtype=np.float32)
    a = np.ascontiguousarray(a, dtype=np.float32)
    U = np.ascontiguousarray(U, dtype=np.float32)
    eye = np.eye(P, dtype=ml_dtypes.bfloat16)
    return [
        {
            "q": q[i * NB : (i + 1) * NB],
            "a": a[i * NB : (i + 1) * NB],
            "U": U,
            "eye": eye,
        }
        for i in range(N_CORES)
    ]


def kernel(q, a, U):
    nc = _get_nc()
    res = run_bass_kernel_spmd(nc, make_in_maps(q, a, U), list(range(N_CORES)))
    q_out = np.concatenate([r["q_out"] for r in res.results], axis=0)
    a_out = np.concatenate([r["a_out"] for r in res.results], axis=0)
    return (q_out, a_out)
